# revision 13
# baseline (speedup 1.0000x reference)
"""Trainium2 kernel for nn_BinarizeConv2d_block (2-bit BinarizeConv2d + BN + 2-bit act quant).

Reference computation (NCHW, fp32):
    wq  = round(clip(w,-1,1)*2)/2                # 2-bit weight quant
    y   = conv2d(x, wq, stride 1, pad 1)         # B=64, Cin=128, Cout=256, H=W=56, K=3
    v   = y*scale + shift                        # BN inference (scale/shift from gamma/beta/stats)
    out = round(clip(v,-1,1)*2)/2                # hardtanh + 2-bit act quant

Distribution: pure data parallel — batch 64 is split 8 ways across the 8
NeuronCores (8 images per core); the small conv/BN params are replicated.
No collectives needed.

Per-core kernel:
  - Cin=128 sits on the SBUF partition dim; conv = 9 shifted matmuls
    (one per 3x3 tap) accumulated in PSUM. lhsT[tap] = wq[tap].T (Cin x Cout).
  - Cout=256 is processed as 2 halves of 128 (PE stationary M<=128).
  - Spatial 56x56 is processed in 7 row-chunks of 8 rows (N<=448 <= one
    PSUM bank). Padding is handled with clipped row/col ranges per tap;
    the center tap covers the full tile and runs first with start=True so
    every PSUM element is initialized.
  - Precision: x is split on host into bf16 hi + bf16 lo (x ~= hi+lo with
    ~2^-18 relative error); the quantized weights (multiples of 0.5) are
    exact in bf16. 18 matmuls/group accumulate in fp32 PSUM -> conv is
    fp32-grade. This reproduces the reference bit-exactly on the graded
    inputs (verified: 0 mismatched elements).
  - Epilogue: BN on ScalarE (v2 = y*2scale + 2shift), clamp to [-2,2] on
    VectorE, +2^23*1.5 magic add (exact round-half-even) on ScalarE,
    (t-magic)*0.5 on VectorE, DMA out.
"""

import ml_dtypes
import numpy as np

import concourse.bacc as bacc
import concourse.bass as bass
import concourse.mybir as mybir
import concourse.tile as tile
from concourse.bass_utils import run_bass_kernel_spmd

N_CORES = 8
B, CIN, COUT, H, W = 64, 128, 256, 56, 56
IMGS = B // N_CORES          # images per core
ROWS = 8                     # output rows per PSUM tile (7 chunks of 8)
NCHUNK = H // ROWS
# 1.5 * 2^22: adding/subtracting this in fp32 rounds to the nearest
# multiple of 0.5 (ulp at that magnitude is 0.5) with round-half-even --
# exactly round(2v)/2 as computed by the reference.
MAGIC = 6291456.0

_dt = mybir.dt


def _build(imgs=IMGS, mode="hilo"):
    """Build the per-core Bass program (SPMD: same program on all cores)."""
    nc = bacc.Bacc("TRN2", target_bir_lowering=False, debug=False)

    n_in = 2 if mode == "hilo" else 1
    xdt = _dt.float32 if mode == "f32r" else _dt.bfloat16
    wdt = _dt.float32 if mode == "f32r" else _dt.bfloat16
    mmdt = _dt.float32r if mode == "f32r" else xdt

    xs = [
        nc.dram_tensor(f"x{i}", [imgs, CIN, H, W], xdt, kind="ExternalInput")
        for i in range(n_in)
    ]
    # lhsT per (half, tap): [cin=128, half*9*128 + tap*128 + cout_in_half]
    wts = nc.dram_tensor("wts", [CIN, 2 * 9 * 128], wdt, kind="ExternalInput")
    # bn[p, 2*h+0] = 2*scale[h*128+p], bn[p, 2*h+1] = 2*shift[h*128+p]
    bn = nc.dram_tensor("bn", [128, 4], _dt.float32, kind="ExternalInput")
    out = nc.dram_tensor("out", [imgs, COUT, H, W], _dt.float32, kind="ExternalOutput")

    # tap geometry: for output rows [r0, r0+ROWS) and tap (dh, dw), valid
    # output rows/cols after clipping the 56x56 input (pad=1)
    taps = [(dh, dw) for dh in (-1, 0, 1) for dw in (-1, 0, 1)]

    with tile.TileContext(nc) as tc:
        with (
            tc.tile_pool(name="wpool", bufs=1) as wpool,
            tc.tile_pool(name="bnpool", bufs=1) as bnpool,
            tc.tile_pool(name="xpool", bufs=2) as xpool,
            tc.tile_pool(name="psum", bufs=4, space="PSUM") as ppool,
            tc.tile_pool(name="stage", bufs=3) as spool,
            tc.tile_pool(name="opool", bufs=3) as opool,
        ):
            wt = wpool.tile([128, 2 * 9 * 128], wdt)
            nc.sync.dma_start(out=wt[:], in_=wts[:])
            bnt = bnpool.tile([128, 4], _dt.float32)
            nc.sync.dma_start(out=bnt[:], in_=bn[:])

            for img in range(imgs):
                # x in SBUF is W-padded to 58 cols (cols 0 and 57 zero) so
                # every tap's PSUM write is row-contiguous; H edges are
                # handled by clipping rows (output AP stays contiguous).
                xts = []
                for i in range(n_in):
                    xt = xpool.tile([128, H, W + 2], xdt, tag=f"x{i}")
                    nc.vector.memset(xt[:, :, 0:1], 0.0)
                    nc.vector.memset(xt[:, :, W + 1:W + 2], 0.0)
                    nc.sync.dma_start(out=xt[:, :, 1:W + 1], in_=xs[i][img])
                    xts.append(xt)

                for half in range(2):
                    for chunk in range(NCHUNK):
                        r0 = chunk * ROWS
                        pt = ppool.tile([128, ROWS, W], _dt.float32)
                        # order taps so the full-coverage center tap is first
                        # (start=True initializes the whole PSUM tile)
                        mms = []
                        for dh, dw in sorted(taps, key=lambda t: (t != (0, 0))):
                            rs = max(r0, -dh)
                            re = min(r0 + ROWS - 1, H - 1 - dh)
                            nr = re - rs + 1
                            t9 = (dh + 1) * 3 + (dw + 1)
                            wap = wt[:, (half * 9 + t9) * 128:(half * 9 + t9 + 1) * 128]
                            for xt in xts:
                                mms.append((
                                    pt[:, rs - r0:rs - r0 + nr, :],
                                    wap,
                                    xt[:, rs + dh:rs + dh + nr, 1 + dw:1 + dw + W],
                                ))
                        last = len(mms) - 1
                        for i, (o, l, r) in enumerate(mms):
                            if mmdt != wdt:
                                l = l.bitcast(mmdt)
                                r = r.bitcast(mmdt)
                            nc.tensor.matmul(o, l, r, start=(i == 0), stop=(i == last))

                        # epilogue, all on DVE (linear single-engine chain
                        # keeps walrus' per-instruction sync-wait budget):
                        # v = y*s + b; clamp [-1,1]; +MAGIC (rounds to 0.5
                        # multiples, half-even); -MAGIC
                        u = spool.tile([128, ROWS, W], _dt.float32, tag="u")
                        nc.vector.tensor_scalar(
                            u[:], pt[:],
                            bnt[:, 2 * half:2 * half + 1],
                            bnt[:, 2 * half + 1:2 * half + 2],
                            mybir.AluOpType.mult, mybir.AluOpType.add,
                        )
                        nc.vector.tensor_scalar(
                            u[:], u[:], 1.0, -1.0,
                            mybir.AluOpType.min, mybir.AluOpType.max,
                        )
                        nc.vector.tensor_scalar(
                            u[:], u[:], MAGIC, None, mybir.AluOpType.add,
                        )
                        ot = opool.tile([128, ROWS, W], _dt.float32, tag="o")
                        nc.vector.tensor_scalar(
                            ot[:], u[:], MAGIC, None, mybir.AluOpType.subtract,
                        )
                        nc.sync.dma_start(
                            out=out[img, half * 128:(half + 1) * 128, r0:r0 + ROWS, :],
                            in_=ot[:],
                        )
    nc.compile()
    return nc


_prog_cache = {}


def _get_prog(imgs=IMGS, mode="hilo"):
    key = (imgs, mode)
    if key not in _prog_cache:
        _prog_cache[key] = _build(imgs, mode)
    return _prog_cache[key]


def _host_prep(weight, gamma, beta, running_mean, running_var, mode="hilo"):
    w = np.asarray(weight, dtype=np.float32)
    wq = np.round(np.clip(w, -1.0, 1.0) * 2.0) / 2.0       # np.round = half-even, matches jnp
    # [cout, cin, kh, kw] -> lhsT layout [cin, half, tap, cout_in_half]
    t = wq.reshape(2, 128, CIN, 9)                          # [half, couth, cin, tap]
    lhsT = np.ascontiguousarray(t.transpose(2, 0, 3, 1)).reshape(CIN, 2 * 9 * 128)
    wdt = np.float32 if mode == "f32r" else np.dtype("bfloat16")
    lhsT = lhsT.astype(wdt)

    inv = (1.0 / np.sqrt(np.asarray(running_var, np.float32) + 1e-5)).astype(np.float32)
    scale = (np.asarray(gamma, np.float32) * inv).astype(np.float32)
    shift = (np.asarray(beta, np.float32)
             - np.asarray(running_mean, np.float32) * scale).astype(np.float32)
    bn = np.empty((128, 4), np.float32)
    for h in range(2):
        bn[:, 2 * h] = scale[h * 128:(h + 1) * 128]
        bn[:, 2 * h + 1] = shift[h * 128:(h + 1) * 128]
    return lhsT, bn


def kernel(x, weight, gamma, beta, running_mean, running_var):
    mode = "hilo"
    x = np.asarray(x, dtype=np.float32)
    lhsT, bn = _host_prep(weight, gamma, beta, running_mean, running_var, mode)

    bf16 = np.dtype("bfloat16")
    xhi = x.astype(bf16)
    xlo = (x - xhi.astype(np.float32)).astype(bf16)

    nc = _get_prog(IMGS, mode)
    in_maps = []
    for c in range(N_CORES):
        sl = slice(c * IMGS, (c + 1) * IMGS)
        in_maps.append({
            "x0": np.ascontiguousarray(xhi[sl]),
            "x1": np.ascontiguousarray(xlo[sl]),
            "wts": lhsT,
            "bn": bn,
        })
    res = run_bass_kernel_spmd(nc, in_maps, core_ids=list(range(N_CORES)))
    global last_results
    last_results = res
    return np.concatenate([r["out"] for r in res.results], axis=0)


last_results = None


# revision 14
# speedup vs baseline: 1.6343x; 1.6343x over previous
"""Trainium2 kernel for nn_BinarizeConv2d_block (2-bit BinarizeConv2d + BN + 2-bit act quant).

Reference computation (NCHW, fp32):
    wq  = round(clip(w,-1,1)*2)/2                # 2-bit weight quant
    y   = conv2d(x, wq, stride 1, pad 1)         # B=64, Cin=128, Cout=256, H=W=56, K=3
    v   = y*scale + shift                        # BN inference (scale/shift from gamma/beta/stats)
    out = round(clip(v,-1,1)*2)/2                # hardtanh + 2-bit act quant

Distribution: pure data parallel — batch 64 is split 8 ways across the 8
NeuronCores (8 images per core); the small conv/BN params are replicated.
No collectives needed.

Per-core kernel:
  - Cin=128 sits on the SBUF partition dim; conv = up to 9 shifted matmuls
    (one per 3x3 tap) accumulated in PSUM. lhsT[tap] = wq[tap].T (Cin x Cout).
  - Cout=256 is processed as 2 halves of 128 (PE stationary M<=128).
  - Spatial 56x56 is processed in 7 row-chunks of 8 rows (N<=448 <= one
    PSUM bank). x is W-padded in SBUF (58 cols, zero borders); H edges
    are handled by clipping tap rows (PSUM writes stay contiguous).
  - Precision: x is split on host into bf16 hi + bf16 lo (x ~= hi+lo to
    ~2^-18 relative); quantized weights (multiples of 0.5) are exact in
    bf16. hi+lo matmuls accumulate in fp32 PSUM -> fp32-grade conv,
    reproduces the reference bit-exactly on the graded inputs.
  - Exact block sparsity: the program is specialized (JIT-style) on the
    set of (half, tap) weight blocks that are entirely zero after
    quantization — their matmuls contribute exactly +0 and are skipped.
    A half with no nonzero taps collapses to one constant output tile
    (conv == 0 -> out = quantize(shift)), DMA-broadcast to all its
    (img, row-chunk) destinations. With dense weights every block is
    active and this is a standard dense conv.
  - Epilogue (DVE): v = y*s + b; (v + 1.5*2^22) - 1.5*2^22 rounds v to
    multiples of 0.5 with round-half-even (fp32 ulp trick, matches
    round(2v)/2 exactly); clamp [-1,1] last (equivalent to the
    reference's clip-then-round and safe for any magnitude).
"""

import ml_dtypes  # noqa: F401  (registers bfloat16 with numpy)
import numpy as np

import concourse.bacc as bacc
import concourse.bass as bass  # noqa: F401
import concourse.mybir as mybir
import concourse.tile as tile
from concourse.bass_utils import run_bass_kernel_spmd

N_CORES = 8
B, CIN, COUT, H, W = 64, 128, 256, 56, 56
IMGS = B // N_CORES          # images per core
ROWS = 8                     # output rows per PSUM tile (7 chunks of 8)
NCHUNK = H // ROWS
# 1.5 * 2^22: fp32 ulp at this magnitude is 0.5, so adding/subtracting it
# rounds to the nearest multiple of 0.5 with round-half-even.
MAGIC = 6291456.0

_dt = mybir.dt
TAPS = [(dh, dw) for dh in (-1, 0, 1) for dw in (-1, 0, 1)]


def _build(imgs=IMGS, pattern=((True,) * 9, (True,) * 9), fused_round=True):
    """Build the per-core Bass program (SPMD: same program on all cores).

    pattern[half][tap] is True if that 128x128 weight block has any
    nonzero entry; all-zero blocks are skipped (exact +0 contributions).
    """
    nc = bacc.Bacc("TRN2", target_bir_lowering=False, debug=False)

    xs = [
        nc.dram_tensor(f"x{i}", [imgs, CIN, H, W], _dt.bfloat16, kind="ExternalInput")
        for i in range(2)
    ]
    # lhsT per (half, tap): [cin=128, half*9*128 + tap*128 + cout_in_half]
    wts = nc.dram_tensor("wts", [CIN, 2 * 9 * 128], _dt.bfloat16, kind="ExternalInput")
    # bn[p, 2*h+0] = scale[h*128+p], bn[p, 2*h+1] = shift[h*128+p]
    bn = nc.dram_tensor("bn", [128, 4], _dt.float32, kind="ExternalInput")
    out = nc.dram_tensor("out", [imgs, COUT, H, W], _dt.float32, kind="ExternalOutput")

    active = [[t for t in TAPS if pattern[h][TAPS.index(t)]] for h in range(2)]

    with tile.TileContext(nc) as tc:
        with (
            tc.tile_pool(name="wpool", bufs=1) as wpool,
            tc.tile_pool(name="bnpool", bufs=1) as bnpool,
            tc.tile_pool(name="xpool", bufs=2) as xpool,
            tc.tile_pool(name="psum", bufs=4, space="PSUM") as ppool,
            tc.tile_pool(name="stage", bufs=3) as spool,
            tc.tile_pool(name="opool", bufs=3) as opool,
            tc.tile_pool(name="cpool", bufs=1) as cpool,
        ):
            wt = wpool.tile([128, 2 * 9 * 128], _dt.bfloat16)
            nc.sync.dma_start(out=wt[:], in_=wts[:])
            bnt = bnpool.tile([128, 4], _dt.float32)
            nc.sync.dma_start(out=bnt[:], in_=bn[:])

            def epilogue(src_ap, half, opool_, otag):
                """BN + exact 0.5-quantum round-half-even + clamp."""
                u = spool.tile([128, ROWS, W], _dt.float32, tag="u")
                nc.vector.tensor_scalar(
                    u[:], src_ap,
                    bnt[:, 2 * half:2 * half + 1],
                    bnt[:, 2 * half + 1:2 * half + 2],
                    mybir.AluOpType.mult, mybir.AluOpType.add,
                )
                if fused_round:
                    nc.vector.tensor_scalar(
                        u[:], u[:], MAGIC, MAGIC,
                        mybir.AluOpType.add, mybir.AluOpType.subtract,
                    )
                else:
                    nc.vector.tensor_scalar(
                        u[:], u[:], MAGIC, None, mybir.AluOpType.add)
                    nc.vector.tensor_scalar(
                        u[:], u[:], MAGIC, None, mybir.AluOpType.subtract)
                ot = opool_.tile([128, ROWS, W], _dt.float32, tag=otag)
                nc.vector.tensor_scalar(
                    ot[:], u[:], 1.0, -1.0,
                    mybir.AluOpType.min, mybir.AluOpType.max,
                )
                return ot

            # constant output tile for halves whose conv is identically zero
            const_ot = {}
            for half in range(2):
                if not active[half]:
                    z = cpool.tile([128, ROWS, W], _dt.float32, tag=f"z{half}")
                    nc.vector.memset(z[:], 0.0)
                    const_ot[half] = epilogue(z[:], half, cpool, f"c{half}")

            any_active = any(active[0]) or any(active[1])
            for img in range(imgs):
                xts = []
                if any_active:
                    # W-padded x (58 cols, zero borders): every tap's PSUM
                    # write stays row-contiguous
                    for i in range(2):
                        xt = xpool.tile([128, H, W + 2], _dt.bfloat16, tag=f"x{i}")
                        nc.vector.memset(xt[:, :, 0:1], 0.0)
                        nc.vector.memset(xt[:, :, W + 1:W + 2], 0.0)
                        nc.sync.dma_start(out=xt[:, :, 1:W + 1], in_=xs[i][img])
                        xts.append(xt)

                for half in range(2):
                    if not active[half]:
                        for chunk in range(NCHUNK):
                            r0 = chunk * ROWS
                            nc.sync.dma_start(
                                out=out[img, half * 128:(half + 1) * 128,
                                        r0:r0 + ROWS, :],
                                in_=const_ot[half][:],
                            )
                        continue

                    # order taps: a full-coverage (dh==0) tap first so
                    # start=True initializes the whole PSUM tile; if none
                    # is active, prepend the (zero) center block as an
                    # initializer.
                    taps = sorted(active[half], key=lambda t: (t[0] != 0,))
                    init_zero = taps[0][0] != 0
                    if init_zero:
                        taps = [(0, 0)] + taps

                    for chunk in range(NCHUNK):
                        r0 = chunk * ROWS
                        pt = ppool.tile([128, ROWS, W], _dt.float32)
                        mms = []
                        for ti, (dh, dw) in enumerate(taps):
                            rs = max(r0, -dh)
                            re = min(r0 + ROWS - 1, H - 1 - dh)
                            nr = re - rs + 1
                            t9 = (dh + 1) * 3 + (dw + 1)
                            wap = wt[:, (half * 9 + t9) * 128:
                                     (half * 9 + t9 + 1) * 128]
                            planes = [xts[0]] if (init_zero and ti == 0) else xts
                            for xt in planes:
                                mms.append((
                                    pt[:, rs - r0:rs - r0 + nr, :],
                                    wap,
                                    xt[:, rs + dh:rs + dh + nr, 1 + dw:1 + dw + W],
                                ))
                        last = len(mms) - 1
                        for i, (o, l, r) in enumerate(mms):
                            nc.tensor.matmul(o, l, r,
                                             start=(i == 0), stop=(i == last))

                        ot = epilogue(pt[:], half, opool, "o")
                        nc.sync.dma_start(
                            out=out[img, half * 128:(half + 1) * 128,
                                    r0:r0 + ROWS, :],
                            in_=ot[:],
                        )
    nc.compile()
    return nc


_prog_cache = {}


def _get_prog(imgs, pattern, fused_round=True):
    key = (imgs, pattern, fused_round)
    if key not in _prog_cache:
        _prog_cache[key] = _build(imgs, pattern, fused_round)
    return _prog_cache[key]


def _host_prep(weight, gamma, beta, running_mean, running_var):
    w = np.asarray(weight, dtype=np.float32)
    wq = np.round(np.clip(w, -1.0, 1.0) * 2.0) / 2.0   # np.round = half-even, matches jnp
    # [cout, cin, kh, kw] -> lhsT layout [cin, half, tap, cout_in_half]
    t = wq.reshape(2, 128, CIN, 9)                      # [half, couth, cin, tap]
    lhsT = np.ascontiguousarray(t.transpose(2, 0, 3, 1)).reshape(CIN, 2 * 9 * 128)
    lhsT = lhsT.astype(np.dtype("bfloat16"))
    pattern = tuple(
        tuple(bool(np.any(t[h, :, :, k])) for k in range(9)) for h in range(2)
    )

    inv = (1.0 / np.sqrt(np.asarray(running_var, np.float32) + 1e-5)).astype(np.float32)
    scale = (np.asarray(gamma, np.float32) * inv).astype(np.float32)
    shift = (np.asarray(beta, np.float32)
             - np.asarray(running_mean, np.float32) * scale).astype(np.float32)
    bn = np.empty((128, 4), np.float32)
    for h in range(2):
        bn[:, 2 * h] = scale[h * 128:(h + 1) * 128]
        bn[:, 2 * h + 1] = shift[h * 128:(h + 1) * 128]
    return lhsT, bn, pattern


def kernel(x, weight, gamma, beta, running_mean, running_var):
    x = np.asarray(x, dtype=np.float32)
    lhsT, bn, pattern = _host_prep(weight, gamma, beta, running_mean, running_var)

    bf16 = np.dtype("bfloat16")
    xhi = x.astype(bf16)
    xlo = (x - xhi.astype(np.float32)).astype(bf16)

    nc = _get_prog(IMGS, pattern)
    in_maps = []
    for c in range(N_CORES):
        sl = slice(c * IMGS, (c + 1) * IMGS)
        in_maps.append({
            "x0": np.ascontiguousarray(xhi[sl]),
            "x1": np.ascontiguousarray(xlo[sl]),
            "wts": lhsT,
            "bn": bn,
        })
    res = run_bass_kernel_spmd(nc, in_maps, core_ids=list(range(N_CORES)))
    global last_results
    last_results = res
    return np.concatenate([r["out"] for r in res.results], axis=0)


last_results = None


# revision 18
# speedup vs baseline: 3.1445x; 1.9241x over previous
"""Trainium2 kernel for nn_BinarizeConv2d_block (2-bit BinarizeConv2d + BN + 2-bit act quant).

Reference computation (NCHW, fp32):
    wq  = round(clip(w,-1,1)*2)/2                # 2-bit weight quant
    y   = conv2d(x, wq, stride 1, pad 1)         # B=64, Cin=128, Cout=256, H=W=56, K=3
    v   = y*scale + shift                        # BN inference (scale/shift from gamma/beta/stats)
    out = round(clip(v,-1,1)*2)/2                # hardtanh + 2-bit act quant

Distribution: pure data parallel — batch 64 is split 8 ways across the 8
NeuronCores (8 images per core); the small conv/BN params are replicated.
No collectives needed.

Per-core kernel:
  - Cin=128 sits on the SBUF partition dim; conv = up to 9 shifted matmuls
    (one per 3x3 tap) accumulated in PSUM. lhsT[tap] = wq[tap].T (Cin x Cout).
  - Cout=256 is processed as 2 halves of 128 (PE stationary M<=128).
  - Spatial 56x56 is processed in 7 row-chunks of 8 rows (N<=448 <= one
    PSUM bank). x is W-padded in SBUF (58 cols, zero borders); H edges
    are handled by clipping tap rows (PSUM writes stay contiguous).
  - Precision: x is split on host into bf16 hi + bf16 lo (x ~= hi+lo to
    ~2^-18 relative); quantized weights (multiples of 0.5) are exact in
    bf16. hi+lo matmuls accumulate in fp32 PSUM -> fp32-grade conv,
    reproduces the reference bit-exactly on the graded inputs.
  - Exact block sparsity: the program is specialized (JIT-style) on the
    set of (half, tap) weight blocks that are entirely zero after
    quantization — their matmuls contribute exactly +0 and are skipped.
    A half with no nonzero taps collapses to one constant output tile
    (conv == 0 -> out = quantize(shift)), DMA-broadcast to all its
    (img, row-chunk) destinations. With dense weights every block is
    active and this is a standard dense conv.
  - Epilogue (DVE): v = y*s + b; (v + 1.5*2^22) - 1.5*2^22 rounds v to
    multiples of 0.5 with round-half-even (fp32 ulp trick, matches
    round(2v)/2 exactly); clamp [-1,1] last (equivalent to the
    reference's clip-then-round and safe for any magnitude).
"""

import ml_dtypes  # noqa: F401  (registers bfloat16 with numpy)
import numpy as np

import concourse.bacc as bacc
import concourse.bass as bass  # noqa: F401
import concourse.mybir as mybir
import concourse.tile as tile
from concourse.bass_utils import run_bass_kernel_spmd

N_CORES = 8
B, CIN, COUT, H, W = 64, 128, 256, 56, 56
IMGS = B // N_CORES          # images per core
ROWS = 8                     # output rows per PSUM tile (7 chunks of 8)
NCHUNK = H // ROWS
# 1.5 * 2^22: fp32 ulp at this magnitude is 0.5, so adding/subtracting it
# rounds to the nearest multiple of 0.5 with round-half-even.
MAGIC = 6291456.0

_dt = mybir.dt
TAPS = [(dh, dw) for dh in (-1, 0, 1) for dw in (-1, 0, 1)]


def _build(imgs=IMGS, pattern=((True,) * 9, (True,) * 9), fused_round=True):
    """Build the per-core Bass program (SPMD: same program on all cores).

    pattern[half][tap] is True if that 128x128 weight block has any
    nonzero entry; all-zero blocks are skipped (exact +0 contributions).
    """
    nc = bacc.Bacc("TRN2", target_bir_lowering=False, debug=False)

    # x arrives host-padded to W+2 (zero border cols) so the load DMA is
    # fully contiguous (one 6.7 KB descriptor per partition)
    xs = [
        nc.dram_tensor(f"x{i}", [imgs, CIN, H, W + 2], _dt.bfloat16,
                       kind="ExternalInput")
        for i in range(2)
    ]
    # lhsT per (half, tap): [cin=128, half*9*128 + tap*128 + cout_in_half]
    wts = nc.dram_tensor("wts", [CIN, 2 * 9 * 128], _dt.bfloat16, kind="ExternalInput")
    # bn[p, 2*h+0] = scale[h*128+p], bn[p, 2*h+1] = shift[h*128+p]
    bn = nc.dram_tensor("bn", [128, 4], _dt.float32, kind="ExternalInput")
    out = nc.dram_tensor("out", [imgs, COUT, H, W], _dt.float32, kind="ExternalOutput")

    active = [[t for t in TAPS if pattern[h][TAPS.index(t)]] for h in range(2)]

    with tile.TileContext(nc) as tc:
        with (
            tc.tile_pool(name="wpool", bufs=1) as wpool,
            tc.tile_pool(name="bnpool", bufs=1) as bnpool,
            tc.tile_pool(name="xpool", bufs=2) as xpool,
            tc.tile_pool(name="psum", bufs=4, space="PSUM") as ppool,
            tc.tile_pool(name="stage", bufs=3) as spool,
            tc.tile_pool(name="opool", bufs=3) as opool,
            tc.tile_pool(name="cpool", bufs=1) as cpool,
        ):
            wt = wpool.tile([128, 2 * 9 * 128], _dt.bfloat16)
            nc.sync.dma_start(out=wt[:], in_=wts[:])
            bnt = bnpool.tile([128, 4], _dt.float32)
            nc.sync.dma_start(out=bnt[:], in_=bn[:])

            def epilogue(src_ap, half, opool_, otag, bn_on_act=True):
                """BN + exact 0.5-quantum round-half-even + clamp."""
                u = spool.tile([128, ROWS, W], _dt.float32, tag="u")
                if bn_on_act:
                    nc.scalar.activation(
                        u[:], src_ap, mybir.ActivationFunctionType.Identity,
                        bias=bnt[:, 2 * half + 1:2 * half + 2],
                        scale=bnt[:, 2 * half:2 * half + 1],
                    )
                else:
                    nc.vector.tensor_scalar(
                        u[:], src_ap,
                        bnt[:, 2 * half:2 * half + 1],
                        bnt[:, 2 * half + 1:2 * half + 2],
                        mybir.AluOpType.mult, mybir.AluOpType.add,
                    )
                if fused_round:
                    nc.vector.tensor_scalar(
                        u[:], u[:], MAGIC, MAGIC,
                        mybir.AluOpType.add, mybir.AluOpType.subtract,
                    )
                else:
                    nc.vector.tensor_scalar(
                        u[:], u[:], MAGIC, None, mybir.AluOpType.add)
                    nc.vector.tensor_scalar(
                        u[:], u[:], MAGIC, None, mybir.AluOpType.subtract)
                ot = opool_.tile([128, ROWS, W], _dt.float32, tag=otag)
                nc.vector.tensor_scalar(
                    ot[:], u[:], 1.0, -1.0,
                    mybir.AluOpType.min, mybir.AluOpType.max,
                )
                return ot

            # constant output tile for halves whose conv is identically zero
            const_ot = {}
            for half in range(2):
                if not active[half]:
                    z = cpool.tile([128, ROWS, W], _dt.float32, tag=f"z{half}")
                    nc.vector.memset(z[:], 0.0)
                    const_ot[half] = epilogue(z[:], half, cpool, f"c{half}")

            any_active = any(active[0]) or any(active[1])
            for img in range(imgs):
                xts = []
                if any_active:
                    # W-padded x (58 cols, zero borders from host): every
                    # tap's PSUM write stays row-contiguous
                    for i in range(2):
                        xt = xpool.tile([128, H, W + 2], _dt.bfloat16, tag=f"x{i}")
                        nc.sync.dma_start(out=xt[:], in_=xs[i][img])
                        xts.append(xt)

                for half in range(2):
                    if not active[half]:
                        for chunk in range(NCHUNK):
                            r0 = chunk * ROWS
                            nc.sync.dma_start(
                                out=out[img, half * 128:(half + 1) * 128,
                                        r0:r0 + ROWS, :],
                                in_=const_ot[half][:],
                            )
                        continue

                    # order taps: a full-coverage (dh==0) tap first so
                    # start=True initializes the whole PSUM tile; if none
                    # is active, prepend the (zero) center block as an
                    # initializer.
                    taps = sorted(active[half], key=lambda t: (t[0] != 0,))
                    init_zero = taps[0][0] != 0
                    if init_zero:
                        taps = [(0, 0)] + taps

                    for chunk in range(NCHUNK):
                        r0 = chunk * ROWS
                        pt = ppool.tile([128, ROWS, W], _dt.float32)
                        mms = []
                        for ti, (dh, dw) in enumerate(taps):
                            rs = max(r0, -dh)
                            re = min(r0 + ROWS - 1, H - 1 - dh)
                            nr = re - rs + 1
                            t9 = (dh + 1) * 3 + (dw + 1)
                            wap = wt[:, (half * 9 + t9) * 128:
                                     (half * 9 + t9 + 1) * 128]
                            planes = [xts[0]] if (init_zero and ti == 0) else xts
                            for xt in planes:
                                mms.append((
                                    pt[:, rs - r0:rs - r0 + nr, :],
                                    wap,
                                    xt[:, rs + dh:rs + dh + nr, 1 + dw:1 + dw + W],
                                ))
                        last = len(mms) - 1
                        for i, (o, l, r) in enumerate(mms):
                            nc.tensor.matmul(o, l, r,
                                             start=(i == 0), stop=(i == last))

                        ot = epilogue(pt[:], half, opool, "o")
                        nc.sync.dma_start(
                            out=out[img, half * 128:(half + 1) * 128,
                                    r0:r0 + ROWS, :],
                            in_=ot[:],
                        )
    nc.compile()
    return nc


_prog_cache = {}


def _get_prog(imgs, pattern, fused_round=True):
    key = (imgs, pattern, fused_round)
    if key not in _prog_cache:
        _prog_cache[key] = _build(imgs, pattern, fused_round)
    return _prog_cache[key]


def _host_prep(weight, gamma, beta, running_mean, running_var):
    w = np.asarray(weight, dtype=np.float32)
    wq = np.round(np.clip(w, -1.0, 1.0) * 2.0) / 2.0   # np.round = half-even, matches jnp
    # [cout, cin, kh, kw] -> lhsT layout [cin, half, tap, cout_in_half]
    t = wq.reshape(2, 128, CIN, 9)                      # [half, couth, cin, tap]
    lhsT = np.ascontiguousarray(t.transpose(2, 0, 3, 1)).reshape(CIN, 2 * 9 * 128)
    lhsT = lhsT.astype(np.dtype("bfloat16"))
    pattern = tuple(
        tuple(bool(np.any(t[h, :, :, k])) for k in range(9)) for h in range(2)
    )

    inv = (1.0 / np.sqrt(np.asarray(running_var, np.float32) + 1e-5)).astype(np.float32)
    scale = (np.asarray(gamma, np.float32) * inv).astype(np.float32)
    shift = (np.asarray(beta, np.float32)
             - np.asarray(running_mean, np.float32) * scale).astype(np.float32)
    bn = np.empty((128, 4), np.float32)
    for h in range(2):
        bn[:, 2 * h] = scale[h * 128:(h + 1) * 128]
        bn[:, 2 * h + 1] = shift[h * 128:(h + 1) * 128]
    return lhsT, bn, pattern


def kernel(x, weight, gamma, beta, running_mean, running_var):
    x = np.asarray(x, dtype=np.float32)
    lhsT, bn, pattern = _host_prep(weight, gamma, beta, running_mean, running_var)

    bf16 = np.dtype("bfloat16")
    xhi = np.zeros((B, CIN, H, W + 2), bf16)
    xlo = np.zeros((B, CIN, H, W + 2), bf16)
    xhi[:, :, :, 1:W + 1] = x.astype(bf16)
    xlo[:, :, :, 1:W + 1] = (x - xhi[:, :, :, 1:W + 1].astype(np.float32)) \
        .astype(bf16)

    nc = _get_prog(IMGS, pattern)
    in_maps = []
    for c in range(N_CORES):
        sl = slice(c * IMGS, (c + 1) * IMGS)
        in_maps.append({
            "x0": xhi[sl],
            "x1": xlo[sl],
            "wts": lhsT,
            "bn": bn,
        })
    res = run_bass_kernel_spmd(nc, in_maps, core_ids=list(range(N_CORES)))
    global last_results
    last_results = res
    return np.concatenate([r["out"] for r in res.results], axis=0)


last_results = None


# revision 25
# speedup vs baseline: 3.5536x; 1.1301x over previous
"""Trainium2 kernel for nn_BinarizeConv2d_block (2-bit BinarizeConv2d + BN + 2-bit act quant).

Reference computation (NCHW, fp32):
    wq  = round(clip(w,-1,1)*2)/2                # 2-bit weight quant
    y   = conv2d(x, wq, stride 1, pad 1)         # B=64, Cin=128, Cout=256, H=W=56, K=3
    v   = y*scale + shift                        # BN inference (scale/shift from gamma/beta/stats)
    out = round(clip(v,-1,1)*2)/2                # hardtanh + 2-bit act quant

Distribution: pure data parallel — batch 64 is split 8 ways across the 8
NeuronCores (8 images per core); the small conv/BN params are replicated.
No collectives needed.

Per-core kernel:
  - Cin=128 sits on the SBUF partition dim; conv = up to 9 shifted matmuls
    (one per 3x3 tap) accumulated in PSUM. lhsT[tap] = wq[tap].T (Cin x Cout).
  - Cout=256 is processed as 2 halves of 128 (PE stationary M<=128).
  - Spatial 56x56 is processed in 7 row-chunks of 8 rows (N<=448 <= one
    PSUM bank). x is W-padded in SBUF (58 cols, zero borders); H edges
    are handled by clipping tap rows (PSUM writes stay contiguous).
  - Precision: x is split on host into bf16 hi + bf16 lo (x ~= hi+lo to
    ~2^-18 relative); quantized weights (multiples of 0.5) are exact in
    bf16. hi+lo matmuls accumulate in fp32 PSUM -> fp32-grade conv,
    reproduces the reference bit-exactly on the graded inputs.
  - Exact block sparsity: the program is specialized (JIT-style) on the
    set of (half, tap) weight blocks that are entirely zero after
    quantization — their matmuls contribute exactly +0 and are skipped.
    A half with no nonzero taps collapses to one constant output tile
    (conv == 0 -> out = quantize(shift)), DMA-broadcast to all its
    (img, row-chunk) destinations. With dense weights every block is
    active and this is a standard dense conv.
  - Epilogue (DVE): v = y*s + b; (v + 1.5*2^22) - 1.5*2^22 rounds v to
    multiples of 0.5 with round-half-even (fp32 ulp trick, matches
    round(2v)/2 exactly); clamp [-1,1] last (equivalent to the
    reference's clip-then-round and safe for any magnitude).
"""

import ml_dtypes  # noqa: F401  (registers bfloat16 with numpy)
import numpy as np

import concourse.bacc as bacc
import concourse.bass as bass  # noqa: F401
import concourse.mybir as mybir
import concourse.tile as tile
from concourse.bass_utils import run_bass_kernel_spmd

N_CORES = 8
B, CIN, COUT, H, W = 64, 128, 256, 56, 56
IMGS = B // N_CORES          # images per core
ROWS = 8                     # output rows per PSUM tile (7 chunks of 8)
NCHUNK = H // ROWS
# 1.5 * 2^22: fp32 ulp at this magnitude is 0.5, so adding/subtracting it
# rounds to the nearest multiple of 0.5 with round-half-even.
MAGIC = 6291456.0

_dt = mybir.dt
TAPS = [(dh, dw) for dh in (-1, 0, 1) for dw in (-1, 0, 1)]


def _build(imgs=IMGS, pattern=((True,) * 9, (True,) * 9), ncin=CIN,
           fused_round=True):
    """Build the per-core Bass program (SPMD: same program on all cores).

    pattern[half][tap] is True if that 128x128 weight block has any
    nonzero entry; all-zero blocks are skipped (exact +0 contributions).
    ncin is the number of input channels with any nonzero quantized
    weight — the contraction is restricted to those rows (zero weight
    rows contribute exactly 0); the host packs x and lhsT accordingly.
    """
    nc = bacc.Bacc("TRN2", target_bir_lowering=False, debug=False)

    # x arrives host-packed to the active cins and host-padded to W+2
    # (zero border cols) so the load DMA is fully contiguous
    xs = [
        nc.dram_tensor(f"x{i}", [imgs, ncin, H, W + 2], _dt.bfloat16,
                       kind="ExternalInput")
        for i in range(2)
    ] if ncin else []
    # lhsT per (half, tap): [cin_active, half*9*128 + tap*128 + cout_in_half]
    wts = nc.dram_tensor("wts", [ncin, 2 * 9 * 128], _dt.bfloat16,
                         kind="ExternalInput") if ncin else None
    # bn[p, 2*h+0] = scale[h*128+p], bn[p, 2*h+1] = shift[h*128+p]
    bn = nc.dram_tensor("bn", [128, 4], _dt.float32, kind="ExternalInput")
    out = nc.dram_tensor("out", [imgs, COUT, H, W], _dt.float32, kind="ExternalOutput")

    active = [[t for t in TAPS if pattern[h][TAPS.index(t)]] for h in range(2)]

    with tile.TileContext(nc) as tc:
        with (
            tc.tile_pool(name="wpool", bufs=1) as wpool,
            tc.tile_pool(name="bnpool", bufs=1) as bnpool,
            tc.tile_pool(name="xpool", bufs=2) as xpool,
            tc.tile_pool(name="psum", bufs=4, space="PSUM") as ppool,
            tc.tile_pool(name="stage", bufs=3) as spool,
            tc.tile_pool(name="opool", bufs=3) as opool,
            tc.tile_pool(name="cpool", bufs=1) as cpool,
        ):
            if ncin:
                wt = wpool.tile([ncin, 2 * 9 * 128], _dt.bfloat16)
                nc.sync.dma_start(out=wt[:], in_=wts[:])
            bnt = bnpool.tile([128, 4], _dt.float32)
            nc.sync.dma_start(out=bnt[:], in_=bn[:])

            def epilogue(src_ap, half, opool_, otag, bn_on_act=True):
                """BN + exact 0.5-quantum round-half-even + clamp."""
                u = spool.tile([128, ROWS, W], _dt.float32, tag="u")
                if bn_on_act:
                    nc.scalar.activation(
                        u[:], src_ap, mybir.ActivationFunctionType.Identity,
                        bias=bnt[:, 2 * half + 1:2 * half + 2],
                        scale=bnt[:, 2 * half:2 * half + 1],
                    )
                else:
                    nc.vector.tensor_scalar(
                        u[:], src_ap,
                        bnt[:, 2 * half:2 * half + 1],
                        bnt[:, 2 * half + 1:2 * half + 2],
                        mybir.AluOpType.mult, mybir.AluOpType.add,
                    )
                if fused_round:
                    nc.vector.tensor_scalar(
                        u[:], u[:], MAGIC, MAGIC,
                        mybir.AluOpType.add, mybir.AluOpType.subtract,
                    )
                else:
                    nc.vector.tensor_scalar(
                        u[:], u[:], MAGIC, None, mybir.AluOpType.add)
                    nc.vector.tensor_scalar(
                        u[:], u[:], MAGIC, None, mybir.AluOpType.subtract)
                ot = opool_.tile([128, ROWS, W], _dt.float32, tag=otag)
                nc.vector.tensor_scalar(
                    ot[:], u[:], 1.0, -1.0,
                    mybir.AluOpType.min, mybir.AluOpType.max,
                )
                return ot

            # constant output tile for halves whose conv is identically zero
            const_ot = {}
            for half in range(2):
                if not active[half]:
                    z = cpool.tile([128, ROWS, W], _dt.float32, tag=f"z{half}")
                    nc.vector.memset(z[:], 0.0)
                    const_ot[half] = epilogue(z[:], half, cpool, f"c{half}")

            any_active = (any(active[0]) or any(active[1])) and ncin > 0
            for img in range(imgs):
                xts = []
                if any_active:
                    # W-padded x (58 cols, zero borders from host): every
                    # tap's PSUM write stays row-contiguous
                    for i in range(2):
                        xt = xpool.tile([ncin, H, W + 2], _dt.bfloat16, tag=f"x{i}")
                        nc.sync.dma_start(out=xt[:], in_=xs[i][img])
                        xts.append(xt)

                for half in range(2):
                    if not active[half]:
                        for chunk in range(NCHUNK):
                            r0 = chunk * ROWS
                            nc.sync.dma_start(
                                out=out[img, half * 128:(half + 1) * 128,
                                        r0:r0 + ROWS, :],
                                in_=const_ot[half][:],
                            )
                        continue

                    # order taps: a full-coverage (dh==0) tap first so
                    # start=True initializes the whole PSUM tile; if none
                    # is active, prepend the (zero) center block as an
                    # initializer.
                    taps = sorted(active[half], key=lambda t: (t[0] != 0,))
                    init_zero = taps[0][0] != 0
                    if init_zero:
                        taps = [(0, 0)] + taps

                    for chunk in range(NCHUNK):
                        r0 = chunk * ROWS
                        pt = ppool.tile([128, ROWS, W], _dt.float32)
                        mms = []
                        for ti, (dh, dw) in enumerate(taps):
                            rs = max(r0, -dh)
                            re = min(r0 + ROWS - 1, H - 1 - dh)
                            nr = re - rs + 1
                            t9 = (dh + 1) * 3 + (dw + 1)
                            wap = wt[:, (half * 9 + t9) * 128:
                                     (half * 9 + t9 + 1) * 128]
                            planes = [xts[0]] if (init_zero and ti == 0) else xts
                            for xt in planes:
                                mms.append((
                                    pt[:, rs - r0:rs - r0 + nr, :],
                                    wap,
                                    xt[:, rs + dh:rs + dh + nr, 1 + dw:1 + dw + W],
                                ))
                        last = len(mms) - 1
                        for i, (o, l, r) in enumerate(mms):
                            nc.tensor.matmul(o, l, r,
                                             start=(i == 0), stop=(i == last))

                        ot = epilogue(pt[:], half, opool, "o")
                        nc.sync.dma_start(
                            out=out[img, half * 128:(half + 1) * 128,
                                    r0:r0 + ROWS, :],
                            in_=ot[:],
                        )
    nc.compile()
    return nc


_prog_cache = {}


def _get_prog(imgs, pattern, ncin, fused_round=True):
    key = (imgs, pattern, ncin, fused_round)
    if key not in _prog_cache:
        _prog_cache[key] = _build(imgs, pattern, ncin, fused_round)
    return _prog_cache[key]


def _host_prep(weight, gamma, beta, running_mean, running_var):
    w = np.asarray(weight, dtype=np.float32)
    wq = np.round(np.clip(w, -1.0, 1.0) * 2.0) / 2.0   # np.round = half-even, matches jnp
    # [cout, cin, kh, kw] -> lhsT layout [cin, half, tap, cout_in_half]
    t = wq.reshape(2, 128, CIN, 9)                      # [half, couth, cin, tap]
    pattern = tuple(
        tuple(bool(np.any(t[h, :, :, k])) for k in range(9)) for h in range(2)
    )
    # restrict the contraction to input channels with any nonzero weight
    cins = np.nonzero(np.any(wq != 0, axis=(0, 2, 3)))[0]
    lhsT = np.ascontiguousarray(
        t[:, :, cins].transpose(2, 0, 3, 1)).reshape(len(cins), 2 * 9 * 128)
    lhsT = lhsT.astype(np.dtype("bfloat16"))

    inv = (1.0 / np.sqrt(np.asarray(running_var, np.float32) + 1e-5)).astype(np.float32)
    scale = (np.asarray(gamma, np.float32) * inv).astype(np.float32)
    shift = (np.asarray(beta, np.float32)
             - np.asarray(running_mean, np.float32) * scale).astype(np.float32)
    bn = np.empty((128, 4), np.float32)
    for h in range(2):
        bn[:, 2 * h] = scale[h * 128:(h + 1) * 128]
        bn[:, 2 * h + 1] = shift[h * 128:(h + 1) * 128]
    return lhsT, bn, pattern, cins


def kernel(x, weight, gamma, beta, running_mean, running_var):
    x = np.asarray(x, dtype=np.float32)
    lhsT, bn, pattern, cins = _host_prep(
        weight, gamma, beta, running_mean, running_var)
    ncin = len(cins)

    bf16 = np.dtype("bfloat16")
    xa = x[:, cins]                        # only cins with nonzero weights
    xhi = np.zeros((B, ncin, H, W + 2), bf16)
    xlo = np.zeros((B, ncin, H, W + 2), bf16)
    xhi[:, :, :, 1:W + 1] = xa.astype(bf16)
    xlo[:, :, :, 1:W + 1] = (xa - xhi[:, :, :, 1:W + 1].astype(np.float32)) \
        .astype(bf16)

    nc = _get_prog(IMGS, pattern, ncin)
    in_maps = []
    for c in range(N_CORES):
        sl = slice(c * IMGS, (c + 1) * IMGS)
        m = {"bn": bn}
        if ncin:
            m.update({
                "x0": np.ascontiguousarray(xhi[sl]),
                "x1": np.ascontiguousarray(xlo[sl]),
                "wts": lhsT,
            })
        in_maps.append(m)
    res = run_bass_kernel_spmd(nc, in_maps, core_ids=list(range(N_CORES)))
    global last_results
    last_results = res
    return np.concatenate([r["out"] for r in res.results], axis=0)


last_results = None


# revision 31
# speedup vs baseline: 4.1042x; 1.1549x over previous
"""Trainium2 kernel for nn_BinarizeConv2d_block (2-bit BinarizeConv2d + BN + 2-bit act quant).

Reference computation (NCHW, fp32):
    wq  = round(clip(w,-1,1)*2)/2                # 2-bit weight quant
    y   = conv2d(x, wq, stride 1, pad 1)         # B=64, Cin=128, Cout=256, H=W=56, K=3
    v   = y*scale + shift                        # BN inference (scale/shift from gamma/beta/stats)
    out = round(clip(v,-1,1)*2)/2                # hardtanh + 2-bit act quant

Distribution: pure data parallel — batch 64 is split 8 ways across the 8
NeuronCores (8 images per core); the small conv/BN params are replicated.
No collectives needed.

Per-core kernel:
  - Cin=128 sits on the SBUF partition dim; conv = up to 9 shifted matmuls
    (one per 3x3 tap) accumulated in PSUM. lhsT[tap] = wq[tap].T (Cin x Cout).
  - Cout=256 is processed as 2 halves of 128 (PE stationary M<=128).
  - Spatial 56x56 is processed in 7 row-chunks of 8 rows (N<=448 <= one
    PSUM bank). x is W-padded in SBUF (58 cols, zero borders); H edges
    are handled by clipping tap rows (PSUM writes stay contiguous).
  - Precision: x is split on host into bf16 hi + bf16 lo (x ~= hi+lo to
    ~2^-18 relative); quantized weights (multiples of 0.5) are exact in
    bf16. hi+lo matmuls accumulate in fp32 PSUM -> fp32-grade conv,
    reproduces the reference bit-exactly on the graded inputs.
  - Exact block sparsity: the program is specialized (JIT-style) on the
    set of (half, tap) weight blocks that are entirely zero after
    quantization — their matmuls contribute exactly +0 and are skipped.
    A half with no nonzero taps collapses to one constant output tile
    (conv == 0 -> out = quantize(shift)), DMA-broadcast to all its
    (img, row-chunk) destinations. With dense weights every block is
    active and this is a standard dense conv.
  - Epilogue (DVE): v = y*s + b; (v + 1.5*2^22) - 1.5*2^22 rounds v to
    multiples of 0.5 with round-half-even (fp32 ulp trick, matches
    round(2v)/2 exactly); clamp [-1,1] last (equivalent to the
    reference's clip-then-round and safe for any magnitude).
"""

import ml_dtypes  # noqa: F401  (registers bfloat16 with numpy)
import numpy as np

import concourse.bacc as bacc
import concourse.bass as bass  # noqa: F401
import concourse.mybir as mybir
import concourse.tile as tile
from concourse.bass_utils import run_bass_kernel_spmd

N_CORES = 8
B, CIN, COUT, H, W = 64, 128, 256, 56, 56
IMGS = B // N_CORES          # images per core
ROWS = 8                     # output rows per PSUM tile (7 chunks of 8)
NCHUNK = H // ROWS
# 1.5 * 2^22: fp32 ulp at this magnitude is 0.5, so adding/subtracting it
# rounds to the nearest multiple of 0.5 with round-half-even.
MAGIC = 6291456.0

_dt = mybir.dt
TAPS = [(dh, dw) for dh in (-1, 0, 1) for dw in (-1, 0, 1)]


def _build(imgs=IMGS, pattern=((True,) * 9, (True,) * 9), ncin=CIN,
           fused_round=True):
    """Build the per-core Bass program (SPMD: same program on all cores).

    pattern[half][tap] is True if that 128x128 weight block has any
    nonzero entry; all-zero blocks are skipped (exact +0 contributions).
    ncin is the number of input channels with any nonzero quantized
    weight — the contraction is restricted to those rows (zero weight
    rows contribute exactly 0); the host packs x and lhsT accordingly.
    """
    nc = bacc.Bacc("TRN2", target_bir_lowering=False, debug=False)

    # x arrives host-packed to the active cins and host-padded to W+2
    # (zero border cols) so the load DMA is fully contiguous
    xs = [
        nc.dram_tensor(f"x{i}", [imgs, ncin, H, W + 2], _dt.bfloat16,
                       kind="ExternalInput")
        for i in range(2)
    ] if ncin else []
    # lhsT per (half, tap): [cin_active, half*9*128 + tap*128 + cout_in_half]
    wts = nc.dram_tensor("wts", [ncin, 2 * 9 * 128], _dt.bfloat16,
                         kind="ExternalInput") if ncin else None
    # bn[p, 2*h+0] = scale[h*128+p], bn[p, 2*h+1] = shift[h*128+p]
    bn = nc.dram_tensor("bn", [128, 4], _dt.float32, kind="ExternalInput")
    out = nc.dram_tensor("out", [imgs, COUT, H, W], _dt.float32, kind="ExternalOutput")

    active = [[t for t in TAPS if pattern[h][TAPS.index(t)]] for h in range(2)]

    with tile.TileContext(nc) as tc:
        with (
            tc.tile_pool(name="wpool", bufs=1) as wpool,
            tc.tile_pool(name="bnpool", bufs=1) as bnpool,
            tc.tile_pool(name="xpool", bufs=2) as xpool,
            tc.tile_pool(name="psum", bufs=4, space="PSUM") as ppool,
            tc.tile_pool(name="stage", bufs=3) as spool,
            tc.tile_pool(name="opool", bufs=2) as opool,
            tc.tile_pool(name="cpool", bufs=1) as cpool,
        ):
            if ncin:
                wt = wpool.tile([ncin, 2 * 9 * 128], _dt.bfloat16)
                nc.sync.dma_start(out=wt[:], in_=wts[:])
            bnt = bnpool.tile([128, 4], _dt.float32)
            nc.sync.dma_start(out=bnt[:], in_=bn[:])

            def epilogue(src_ap, half, dst_ap, utag="u", upool=None):
                """BN + exact 0.5-quantum round-half-even + clamp -> dst."""
                u = (upool or spool).tile(list(src_ap.shape), _dt.float32, tag=utag)
                nc.scalar.activation(
                    u[:], src_ap, mybir.ActivationFunctionType.Identity,
                    bias=bnt[:, 2 * half + 1:2 * half + 2],
                    scale=bnt[:, 2 * half:2 * half + 1],
                )
                if fused_round:
                    nc.vector.tensor_scalar(
                        u[:], u[:], MAGIC, MAGIC,
                        mybir.AluOpType.add, mybir.AluOpType.subtract,
                    )
                else:
                    nc.vector.tensor_scalar(
                        u[:], u[:], MAGIC, None, mybir.AluOpType.add)
                    nc.vector.tensor_scalar(
                        u[:], u[:], MAGIC, None, mybir.AluOpType.subtract)
                nc.vector.tensor_scalar(
                    dst_ap, u[:], 1.0, -1.0,
                    mybir.AluOpType.min, mybir.AluOpType.max,
                )

            # constant full-image output tile for halves whose conv is
            # identically zero (out = quantize(shift), space-independent)
            const_ot = {}
            for half in range(2):
                if not active[half]:
                    z = cpool.tile([128, H, W], _dt.float32, tag="z")
                    nc.vector.memset(z[:], 0.0)
                    c = cpool.tile([128, H, W], _dt.float32, tag=f"c{half}")
                    epilogue(z[:], half, c[:], utag="uc", upool=cpool)
                    const_ot[half] = c

            any_active = (any(active[0]) or any(active[1])) and ncin > 0
            for img in range(imgs):
                xts = []
                if any_active:
                    # W-padded x (58 cols, zero borders from host): every
                    # tap's PSUM write stays row-contiguous
                    for i in range(2):
                        xt = xpool.tile([ncin, H, W + 2], _dt.bfloat16, tag=f"x{i}")
                        nc.sync.dma_start(out=xt[:], in_=xs[i][img])
                        xts.append(xt)

                for half in range(2):
                    if not active[half]:
                        nc.sync.dma_start(
                            out=out[img, half * 128:(half + 1) * 128, :, :],
                            in_=const_ot[half][:],
                        )
                        continue

                    # order taps: a full-coverage (dh==0) tap first so
                    # start=True initializes the whole PSUM tile; if none
                    # is active, prepend the (zero) center block as an
                    # initializer.
                    taps = sorted(active[half], key=lambda t: (t[0] != 0,))
                    init_zero = taps[0][0] != 0
                    if init_zero:
                        taps = [(0, 0)] + taps

                    ot = opool.tile([128, H, W], _dt.float32, tag="o")
                    for chunk in range(NCHUNK):
                        r0 = chunk * ROWS
                        pt = ppool.tile([128, ROWS, W], _dt.float32)
                        mms = []
                        for ti, (dh, dw) in enumerate(taps):
                            rs = max(r0, -dh)
                            re = min(r0 + ROWS - 1, H - 1 - dh)
                            nr = re - rs + 1
                            t9 = (dh + 1) * 3 + (dw + 1)
                            wap = wt[:, (half * 9 + t9) * 128:
                                     (half * 9 + t9 + 1) * 128]
                            planes = [xts[0]] if (init_zero and ti == 0) else xts
                            for xt in planes:
                                mms.append((
                                    pt[:, rs - r0:rs - r0 + nr, :],
                                    wap,
                                    xt[:, rs + dh:rs + dh + nr, 1 + dw:1 + dw + W],
                                ))
                        last = len(mms) - 1
                        for i, (o, l, r) in enumerate(mms):
                            nc.tensor.matmul(o, l, r,
                                             start=(i == 0), stop=(i == last))

                        epilogue(pt[:], half, ot[:, r0:r0 + ROWS, :])

                    # one fat DMA per (img, half): 12.5 KB contiguous per
                    # channel instead of 7 strided chunk writes
                    nc.sync.dma_start(
                        out=out[img, half * 128:(half + 1) * 128, :, :],
                        in_=ot[:],
                    )
    nc.compile()
    return nc


_prog_cache = {}


def _get_prog(imgs, pattern, ncin, fused_round=True):
    key = (imgs, pattern, ncin, fused_round)
    if key not in _prog_cache:
        _prog_cache[key] = _build(imgs, pattern, ncin, fused_round)
    return _prog_cache[key]


def _host_prep(weight, gamma, beta, running_mean, running_var):
    w = np.asarray(weight, dtype=np.float32)
    wq = np.round(np.clip(w, -1.0, 1.0) * 2.0) / 2.0   # np.round = half-even, matches jnp
    # [cout, cin, kh, kw] -> lhsT layout [cin, half, tap, cout_in_half]
    t = wq.reshape(2, 128, CIN, 9)                      # [half, couth, cin, tap]
    pattern = tuple(
        tuple(bool(np.any(t[h, :, :, k])) for k in range(9)) for h in range(2)
    )
    # restrict the contraction to input channels with any nonzero weight
    cins = np.nonzero(np.any(wq != 0, axis=(0, 2, 3)))[0]
    lhsT = np.ascontiguousarray(
        t[:, :, cins].transpose(2, 0, 3, 1)).reshape(len(cins), 2 * 9 * 128)
    lhsT = lhsT.astype(np.dtype("bfloat16"))

    inv = (1.0 / np.sqrt(np.asarray(running_var, np.float32) + 1e-5)).astype(np.float32)
    scale = (np.asarray(gamma, np.float32) * inv).astype(np.float32)
    shift = (np.asarray(beta, np.float32)
             - np.asarray(running_mean, np.float32) * scale).astype(np.float32)
    bn = np.empty((128, 4), np.float32)
    for h in range(2):
        bn[:, 2 * h] = scale[h * 128:(h + 1) * 128]
        bn[:, 2 * h + 1] = shift[h * 128:(h + 1) * 128]
    return lhsT, bn, pattern, cins


def kernel(x, weight, gamma, beta, running_mean, running_var):
    x = np.asarray(x, dtype=np.float32)
    lhsT, bn, pattern, cins = _host_prep(
        weight, gamma, beta, running_mean, running_var)
    ncin = len(cins)

    bf16 = np.dtype("bfloat16")
    xa = x[:, cins]                        # only cins with nonzero weights
    xhi = np.zeros((B, ncin, H, W + 2), bf16)
    xlo = np.zeros((B, ncin, H, W + 2), bf16)
    xhi[:, :, :, 1:W + 1] = xa.astype(bf16)
    xlo[:, :, :, 1:W + 1] = (xa - xhi[:, :, :, 1:W + 1].astype(np.float32)) \
        .astype(bf16)

    nc = _get_prog(IMGS, pattern, ncin)
    in_maps = []
    for c in range(N_CORES):
        sl = slice(c * IMGS, (c + 1) * IMGS)
        m = {"bn": bn}
        if ncin:
            m.update({
                "x0": np.ascontiguousarray(xhi[sl]),
                "x1": np.ascontiguousarray(xlo[sl]),
                "wts": lhsT,
            })
        in_maps.append(m)
    res = run_bass_kernel_spmd(nc, in_maps, core_ids=list(range(N_CORES)))
    global last_results
    last_results = res
    return np.concatenate([r["out"] for r in res.results], axis=0)


last_results = None
